# revision 64
# baseline (speedup 1.0000x reference)
"""Trainium2 8-core attention kernel for nn_Attention_14104672600564.

Problem: x[4,128,64,64] f32; wq/wk/wv/wo [128,128]; bo[128].
  per (b,h): sim = (wq x)^T (wk x) * d^-.5 ; attn = softmax(sim) ; out_h = attn @ (wv x)^T
  out = wo @ concat_h(out_h^T) + bo

Sharding: 16 independent (batch, head) attention problems -> 8 cores; each core
gets one batch and one head-pair. Each core computes its partial output
wo[:, headslice] @ heads_out [128, 4096]; the host unshards by summing the two
cores of each batch.

Perf design notes:
  - All matmuls bf16 (1 cyc/row; fp32 would be 4), accumulation fp32 in PSUM.
  - EVERY matmul is padded to untiled (128,128) PE mode (K and M padded to >64
    with zero rows/cols). Mixing PE tile modes forces a drain between
    matmuls: measured 630ns/matmul (isolated-cold) vs ~215ns pipelined.
  - softmax has no max-subtraction: |sim| < ~0.5 by construction.
  - exp runs on ACT at FD=1024 ([128,1024] PSUM->SBUF bf16), the intended
    bottleneck engine (~285us/core).
  - The AV matmul's stationary is the V^T block laced with ones columns, so
    output row 0 is the softmax denominator; reciprocal on DVE; broadcast of
    the reciprocal via a [128,128]-padded fp32 matmul whose stationary has a
    single row of ones.
"""

import sys

sys.path.insert(0, "/opt/trn_rl_repo")

import numpy as np
import ml_dtypes

import concourse.bass as bass
import concourse.bacc as bacc
import concourse.tile as tile
from concourse import mybir
import concourse.bass_utils as _bass_utils
from concourse.bass_utils import run_bass_kernel_spmd



BF16 = mybir.dt.bfloat16
F32 = mybir.dt.float32

HEADS = 4
DH = 32  # dim per head
C = 128  # channels
N = 4096  # tokens (64*64)
IC = 512  # i-chunk
NICH = N // IC  # 8
JS = 128  # j-strip
NJS = N // JS  # 32
VBLK = 2 * (DH + 1)  # 66: [1|Vh0|1|Vh1] per j-strip
VTW = VBLK * NJS + 33 + 128  # vt width incl. padding for the 128-wide lhsT AP

_last_results = None  # test harness pokes this for exec_time_ns / profile


def _build():
    nc = bacc.Bacc(None, target_bir_lowering=False)
    xt_d = nc.declare_dram_parameter("xt", [C, N], BF16, isOutput=False)
    wqkT_d = nc.declare_dram_parameter("wqkT", [C, 512], BF16, isOutput=False)
    wvT_d = nc.declare_dram_parameter("wvT", [C, VBLK], BF16, isOutput=False)
    woT_d = nc.declare_dram_parameter("woT", [C, 256], BF16, isOutput=False)
    out_d = nc.declare_dram_parameter("out", [C, N], F32, isOutput=True)
    recd = nc.dram_tensor("recd", [1, 2 * N], F32)  # reciprocal bounce for bcast

    EXP = mybir.ActivationFunctionType.Exp

    with tile.TileContext(nc) as tc:
        with (
            tc.tile_pool(name="singles", bufs=1) as singles,
            tc.tile_pool(name="pts", bufs=8) as pts,
            tc.tile_pool(name="simpool", bufs=3, space="PSUM") as simpool,
            tc.tile_pool(name="avpool", bufs=1, space="PSUM") as avpool,
        ):
            xt_s = singles.tile([C, N], BF16)
            wqkT_s = singles.tile([C, 512], BF16)
            wvT_s = singles.tile([C, VBLK], BF16)
            woT_s = singles.tile([C, 256], BF16)
            qt0 = singles.tile([C, N], BF16)  # head0 Q: rows 0-31, rest zero
            kt0 = singles.tile([C, N], BF16)  # head0 K
            qt1 = singles.tile([C, N], BF16)  # head1 Q
            kt1 = singles.tile([C, N], BF16)  # head1 K
            # one vt tile per j-strip: [1|Vh0|1|Vh1|zeros] padded to 161 cols
            # so both heads' 128-wide lhsT APs stay in untiled (128,128) mode,
            # and each AV matmul depends on exactly one V-projection.
            vts = [
                singles.tile([C, 161], BF16, tag=f"vt{j}", name=f"vt{j}")
                for j in range(NJS)
            ]
            avu = singles.tile([DH + 1, 2 * N], F32)
            avn = singles.tile([C, 2 * N], BF16)  # rows 33-127 zero
            rec = singles.tile([1, 2 * N], F32)  # 1/denom (full precision)
            rscr = singles.tile([1, IC], F32)  # reciprocal NR scratch
            rec_b = singles.tile([C, 2 * N], BF16)  # row 0 bf16 recips, rest zero
            ones_s = singles.tile([C, C], BF16)  # row 0 ones, rest zero
            outs = singles.tile([C, N], F32)

            nc.sync.dma_start(out=wqkT_s[:], in_=wqkT_d[:])
            nc.sync.dma_start(out=wvT_s[:], in_=wvT_d[:])
            nc.sync.dma_start(out=woT_s[:], in_=woT_d[:])
            for j in range(NJS):
                nc.vector.memset(vts[j][:], 0.0)
            nc.vector.memset(avn[:], 0.0)
            nc.vector.memset(rec_b[:], 0.0)
            nc.vector.memset(ones_s[:], 0.0)
            nc.vector.memset(ones_s[0:1, :], 1.0)
            for ic in range(NICH):
                nc.sync.dma_start(
                    out=xt_s[:, IC * ic : IC * (ic + 1)],
                    in_=xt_d[:, IC * ic : IC * (ic + 1)],
                )

            # ---- QK projection. wqkT is host-padded to [128, 512] with each
            # head-tensor's 32 columns at a 128-col stride and zeros elsewhere,
            # so every stationary slice is [W(32)|0(96)]: out rows 32-127 are
            # genuine zeros and the full [128,512] PSUM block lands in the qk
            # tile with no separate zero-fill. Evacuation copies run on ACT
            # (idle during the prologue; DVE was the prologue bottleneck).
            # Head0's copies go to ACT (fast prologue, exp stream starts right
            # after); head1's go to DVE and drain under head0's main loop.
            qts = [qt0, qt1]
            kts = [kt0, kt1]

            def qk_proj_one(h, ic, copy_q, copy_k):
                    ps = simpool.tile([128, 1024], F32, tag="sim")
                    for half in range(2):  # 0: Q, 1: K
                        c = 2 * half + h
                        nc.tensor.matmul(
                            ps[:, IC * half : IC * (half + 1)],
                            lhsT=wqkT_s[:, 128 * c : 128 * (c + 1)],
                            rhs=xt_s[:, IC * ic : IC * (ic + 1)],
                            start=True,
                            stop=True,
                        )
                    for half, eng, dst in ((0, copy_q, qts[h]), (1, copy_k, kts[h])):
                        eng(
                            dst[:, IC * ic : IC * (ic + 1)],
                            ps[:, IC * half : IC * (half + 1)],
                        )

            # head0 first (ACT copies: the exp stream starts right after);
            # head1's projection trickles into head0's second i-chunk pair.
            for ic in range(NICH):
                qk_proj_one(0, ic, nc.scalar.copy, nc.scalar.copy)

            # ---- V^T projection into the ones-laced layout.
            # wvT is host-padded to 66 cols with zeros at cols 0 and 33; the
            # projection writes [junk|Vh0|junk|Vh1] to PSUM, DVE memsets the
            # two junk columns to 1.0, then one contiguous copy (on ACT) lands
            # the whole block.
            def v_proj_one(jc):
                pv = simpool.tile([128, 1024], F32, tag="sim")
                nc.tensor.matmul(
                    pv[:, 0:VBLK],
                    lhsT=xt_s[:, JS * jc : JS * (jc + 1)],
                    rhs=wvT_s[:],
                    start=True,
                    stop=True,
                )
                nc.vector.memset(pv[:, 0:1], 1.0)
                nc.vector.memset(pv[:, 33:34], 1.0)
                nc.scalar.copy(vts[jc][:, 0:VBLK], pv[:, 0:VBLK])

            VLEAD = 6
            for jc in range(VLEAD):  # only the first strips gate the stream
                v_proj_one(jc)

            # ---- main attention loops (heads sequential).
            # Groups are (j-strip, i-chunk-pair): the two sim matmuls share one
            # stationary (the K strip) and the two AV matmuls share another
            # (the vt block), so each LDWEIGHTS serves two matmuls and
            # same-weight matmuls pipeline back-to-back on the PE.
            for h in range(2):
                qt, kt = qts[h], kts[h]
                hoff = N * h  # free offset into avu/avn/rec for this head
                for ip in range(NICH // 2):
                    ica, icb = 2 * ip, 2 * ip + 1
                    ava = avpool.tile([C, IC], F32, tag="av_a")
                    avb = avpool.tile([C, IC], F32, tag="av_b")
                    for js in range(NJS):
                        if h == 0 and ip == 0 and js < NJS - VLEAD:
                            v_proj_one(js + VLEAD)
                        if h == 0 and ip in (1, 2) and js % 8 == 0:
                            qk_proj_one(
                                1,
                                4 * (ip - 1) + js // 8,
                                nc.vector.tensor_copy,
                                nc.vector.tensor_copy,
                            )
                        sim = simpool.tile([128, 1024], F32, tag="sim")
                        for t, ic in enumerate((ica, icb)):
                            nc.tensor.matmul(
                                sim[:, IC * t : IC * (t + 1)],
                                lhsT=kt[:, JS * js : JS * (js + 1)],
                                rhs=qt[:, IC * ic : IC * (ic + 1)],
                                start=True,
                                stop=True,
                            )
                        pt = pts.tile([128, 1024], BF16, tag="pt")
                        nc.scalar.activation(pt[:], sim[:], EXP)
                        for t, av in enumerate((ava, avb)):
                            nc.tensor.matmul(
                                av[:],
                                lhsT=vts[js][:, 33 * h : 33 * h + 128],
                                rhs=pt[:, IC * t : IC * (t + 1)],
                                start=(js == 0),
                                stop=(js == NJS - 1),
                            )
                    # ---- per-(head, i-chunk) epilogue
                    for ic, av in ((ica, ava), (icb, avb)):
                        sl = slice(hoff + IC * ic, hoff + IC * (ic + 1))
                        nc.vector.tensor_copy(avu[:, sl], av[0 : DH + 1, :])
                        nc.vector.reciprocal_approx_accurate(
                            rec[0:1, sl], avu[0:1, sl], scratch=rscr[0:1, :]
                        )
                        # broadcast 1/denom across partitions with a quick
                        # ones-row matmul (shorter dependency chain than the
                        # DRAM round-trip, so the borrowed PSUM slots in the
                        # h1 epilogues free faster)
                        nc.vector.tensor_copy(rec_b[0:1, sl], rec[0:1, sl])
                        pb = simpool.tile([128, 1024], F32, tag="sim", name="pb")
                        nc.tensor.matmul(
                            pb[:, 0:IC],
                            lhsT=ones_s[:],
                            rhs=rec_b[:, sl],
                            start=True,
                            stop=True,
                        )
                        nc.vector.tensor_mul(
                            avn[0 : DH + 1, sl], avu[:, sl], pb[0 : DH + 1, 0:IC]
                        )
                        if h == 1:
                            po = simpool.tile([128, 1024], F32, tag="sim")
                            nc.tensor.matmul(
                                po[:, 0:IC],
                                lhsT=woT_s[:, 0:128],
                                rhs=avn[:, IC * ic : IC * (ic + 1)],
                                start=True,
                                stop=False,
                            )
                            nc.tensor.matmul(
                                po[:, 0:IC],
                                lhsT=woT_s[:, 128:256],
                                rhs=avn[:, N + IC * ic : N + IC * (ic + 1)],
                                start=False,
                                stop=True,
                            )
                            # bias is folded into the projection (avn row 33
                            # is all-ones, woT row 33 of block 0 is bo)
                            nc.vector.tensor_copy(
                                outs[:, IC * ic : IC * (ic + 1)], po[:, 0:IC]
                            )
                            nc.sync.dma_start(
                                out=out_d[:, IC * ic : IC * (ic + 1)],
                                in_=outs[:, IC * ic : IC * (ic + 1)],
                            )
    nc.finalize()
    return nc


_nc_cache = None


def _get_nc():
    global _nc_cache
    if _nc_cache is None:
        _nc_cache = _build()
    return _nc_cache


def make_in_maps(x, wq, wk, wv, wo, bo):
    b = 4
    xt = np.asarray(x, np.float32).reshape(b, C, N)
    wq = np.asarray(wq, np.float32)
    wk = np.asarray(wk, np.float32)
    wv = np.asarray(wv, np.float32)
    wo = np.asarray(wo, np.float32)
    bo = np.asarray(bo, np.float32)
    scale = DH ** (-0.5)

    def bf(a):
        return np.ascontiguousarray(a.astype(ml_dtypes.bfloat16))

    in_maps = []
    for core in range(8):
        bi, hp = core // 2, core % 2
        wq2 = wq[64 * hp : 64 * hp + 64] * scale
        wk2 = wk[64 * hp : 64 * hp + 64]
        wv2 = wv[64 * hp : 64 * hp + 64]
        wqkT = np.zeros((C, 512), np.float32)
        wqkT[:, 0:32] = wq2.T[:, 0:32]  # Qh0
        wqkT[:, 128:160] = wq2.T[:, 32:64]  # Qh1
        wqkT[:, 256:288] = wk2.T[:, 0:32]  # Kh0
        wqkT[:, 384:416] = wk2.T[:, 32:64]  # Kh1
        wvT = np.zeros((C, VBLK), np.float32)  # cols 0,33 stay 0 (psum memset->1)
        wvT[:, 1:33] = wv2.T[:, 0:32]
        wvT[:, 34:66] = wv2.T[:, 32:64]
        woT = np.zeros((C, 256), np.float32)
        woT[1:33, 0:128] = wo[:, 64 * hp : 64 * hp + 32].T
        woT[1:33, 128:256] = wo[:, 64 * hp + 32 : 64 * hp + 64].T
        if hp == 0:
            woT[0, 0:128] = bo  # bias rides avn row 0 (= denom/denom = 1)
        in_maps.append(
            {
                "xt": bf(xt[bi]),
                "wqkT": bf(wqkT),
                "wvT": bf(wvT),
                "woT": bf(woT),
            }
        )
    return in_maps


def kernel(x, wq, wk, wv, wo, bo):
    global _last_results
    in_maps = make_in_maps(x, wq, wk, wv, wo, bo)
    nc = _get_nc()
    res = run_bass_kernel_spmd(nc, in_maps, core_ids=list(range(8)))
    _last_results = res
    outs = res.results
    out = np.zeros((4, C, N), np.float32)
    for bi in range(4):
        out[bi] = np.asarray(outs[2 * bi]["out"], np.float32) + np.asarray(
            outs[2 * bi + 1]["out"], np.float32
        )
    return out.reshape(4, C, 64, 64)


# revision 65
# speedup vs baseline: 1.1124x; 1.1124x over previous
"""Trainium2 8-core attention kernel for nn_Attention_14104672600564.

Problem: x[4,128,64,64] f32; wq/wk/wv/wo [128,128]; bo[128].
  per (b,h): sim = (wq x)^T (wk x) * d^-.5 ; attn = softmax(sim) ; out_h = attn @ (wv x)^T
  out = wo @ concat_h(out_h^T) + bo

Sharding: 16 independent (batch, head) attention problems -> 8 cores; each core
gets one batch and one head-pair. Each core computes its partial output
wo[:, headslice] @ heads_out [128, 4096]; the host unshards by summing the two
cores of each batch.

Perf design notes:
  - All matmuls bf16 (1 cyc/row; fp32 would be 4), accumulation fp32 in PSUM.
  - EVERY matmul is padded to untiled (128,128) PE mode (K and M padded to >64
    with zero rows/cols). Mixing PE tile modes forces a drain between
    matmuls: measured 630ns/matmul (isolated-cold) vs ~215ns pipelined.
  - softmax has no max-subtraction: |sim| < ~0.5 by construction.
  - exp runs on ACT at FD=1024 ([128,1024] PSUM->SBUF bf16), the intended
    bottleneck engine (~285us/core).
  - The AV matmul's stationary is the V^T block laced with ones columns, so
    output row 0 is the softmax denominator; reciprocal on DVE; broadcast of
    the reciprocal via a [128,128]-padded fp32 matmul whose stationary has a
    single row of ones.
"""

import sys

sys.path.insert(0, "/opt/trn_rl_repo")

import numpy as np
import ml_dtypes

import concourse.bass as bass
import concourse.bacc as bacc
import concourse.tile as tile
from concourse import mybir
import concourse.bass_utils as _bass_utils
from concourse.bass_utils import run_bass_kernel_spmd



BF16 = mybir.dt.bfloat16
F32 = mybir.dt.float32

HEADS = 4
DH = 32  # dim per head
C = 128  # channels
N = 4096  # tokens (64*64)
IC = 512  # i-chunk
NICH = N // IC  # 8
JS = 128  # j-strip
NJS = N // JS  # 32
VBLK = 2 * (DH + 1)  # 66: [1|Vh0|1|Vh1] per j-strip
VTW = VBLK * NJS + 33 + 128  # vt width incl. padding for the 128-wide lhsT AP

_last_results = None  # test harness pokes this for exec_time_ns / profile


def _build():
    nc = bacc.Bacc(None, target_bir_lowering=False)
    xt_d = nc.declare_dram_parameter("xt", [C, N], BF16, isOutput=False)
    wqkT_d = nc.declare_dram_parameter("wqkT", [C, 512], BF16, isOutput=False)
    wvT_d = nc.declare_dram_parameter("wvT", [C, VBLK], BF16, isOutput=False)
    woT_d = nc.declare_dram_parameter("woT", [C, 256], BF16, isOutput=False)
    out_d = nc.declare_dram_parameter("out", [C, N], F32, isOutput=True)
    recd = nc.dram_tensor("recd", [1, 2 * N], F32)  # reciprocal bounce for bcast

    EXP = mybir.ActivationFunctionType.Exp

    with tile.TileContext(nc) as tc:
        with (
            tc.tile_pool(name="singles", bufs=1) as singles,
            tc.tile_pool(name="pts", bufs=8) as pts,
            tc.tile_pool(name="simpool", bufs=3, space="PSUM") as simpool,
            tc.tile_pool(name="avpool", bufs=1, space="PSUM") as avpool,
        ):
            xt_s = singles.tile([C, N], BF16)
            wqkT_s = singles.tile([C, 512], BF16)
            wvT_s = singles.tile([C, VBLK], BF16)
            woT_s = singles.tile([C, 256], BF16)
            qt0 = singles.tile([C, N], BF16)  # head0 Q: rows 0-31, rest zero
            kt0 = singles.tile([C, N], BF16)  # head0 K
            qt1 = singles.tile([C, N], BF16)  # head1 Q
            kt1 = singles.tile([C, N], BF16)  # head1 K
            # one vt tile per j-strip: [1|Vh0|1|Vh1|zeros] padded to 161 cols
            # so both heads' 128-wide lhsT APs stay in untiled (128,128) mode,
            # and each AV matmul depends on exactly one V-projection.
            vts = [
                singles.tile([C, 161], BF16, tag=f"vt{j}", name=f"vt{j}")
                for j in range(NJS)
            ]
            avu = singles.tile([DH + 1, 2 * N], F32)
            avn = singles.tile([C, 2 * N], BF16)  # rows 33-127 zero
            rec = singles.tile([1, 2 * N], F32)  # 1/denom (full precision)
            rscr = singles.tile([1, IC], F32)  # reciprocal NR scratch
            bc = singles.tile([DH + 1, N], F32)  # broadcast recips (per-ic reuse)
            outs = singles.tile([C, N], F32)

            nc.sync.dma_start(out=wqkT_s[:], in_=wqkT_d[:])
            nc.sync.dma_start(out=wvT_s[:], in_=wvT_d[:])
            nc.sync.dma_start(out=woT_s[:], in_=woT_d[:])
            for j in range(NJS):
                nc.vector.memset(vts[j][:], 0.0)
            nc.vector.memset(avn[:], 0.0)
            for ic in range(NICH):
                nc.sync.dma_start(
                    out=xt_s[:, IC * ic : IC * (ic + 1)],
                    in_=xt_d[:, IC * ic : IC * (ic + 1)],
                )

            # ---- QK projection. wqkT is host-padded to [128, 512] with each
            # head-tensor's 32 columns at a 128-col stride and zeros elsewhere,
            # so every stationary slice is [W(32)|0(96)]: out rows 32-127 are
            # genuine zeros and the full [128,512] PSUM block lands in the qk
            # tile with no separate zero-fill. Evacuation copies run on ACT
            # (idle during the prologue; DVE was the prologue bottleneck).
            # Head0's copies go to ACT (fast prologue, exp stream starts right
            # after); head1's go to DVE and drain under head0's main loop.
            qts = [qt0, qt1]
            kts = [kt0, kt1]

            def qk_proj_one(h, ic, copy_q, copy_k):
                    ps = simpool.tile([128, 1024], F32, tag="sim")
                    for half in range(2):  # 0: Q, 1: K
                        c = 2 * half + h
                        nc.tensor.matmul(
                            ps[:, IC * half : IC * (half + 1)],
                            lhsT=wqkT_s[:, 128 * c : 128 * (c + 1)],
                            rhs=xt_s[:, IC * ic : IC * (ic + 1)],
                            start=True,
                            stop=True,
                        )
                    for half, eng, dst in ((0, copy_q, qts[h]), (1, copy_k, kts[h])):
                        eng(
                            dst[:, IC * ic : IC * (ic + 1)],
                            ps[:, IC * half : IC * (half + 1)],
                        )

            # head0 first (ACT copies: the exp stream starts right after);
            # head1's projection trickles into head0's second i-chunk pair.
            for ic in range(NICH):
                qk_proj_one(0, ic, nc.scalar.copy, nc.scalar.copy)

            # ---- V^T projection into the ones-laced layout.
            # wvT is host-padded to 66 cols with zeros at cols 0 and 33; the
            # projection writes [junk|Vh0|junk|Vh1] to PSUM, DVE memsets the
            # two junk columns to 1.0, then one contiguous copy (on ACT) lands
            # the whole block.
            def v_proj_one(jc):
                pv = simpool.tile([128, 1024], F32, tag="sim")
                nc.tensor.matmul(
                    pv[:, 0:VBLK],
                    lhsT=xt_s[:, JS * jc : JS * (jc + 1)],
                    rhs=wvT_s[:],
                    start=True,
                    stop=True,
                )
                nc.vector.memset(pv[:, 0:1], 1.0)
                nc.vector.memset(pv[:, 33:34], 1.0)
                nc.scalar.copy(vts[jc][:, 0:VBLK], pv[:, 0:VBLK])

            VLEAD = 6
            for jc in range(VLEAD):  # only the first strips gate the stream
                v_proj_one(jc)

            # ---- main attention loops (heads sequential).
            # Groups are (j-strip, i-chunk-pair): the two sim matmuls share one
            # stationary (the K strip) and the two AV matmuls share another
            # (the vt block), so each LDWEIGHTS serves two matmuls and
            # same-weight matmuls pipeline back-to-back on the PE.
            for h in range(2):
                qt, kt = qts[h], kts[h]
                hoff = N * h  # free offset into avu/avn/rec for this head
                for ip in range(NICH // 2):
                    ica, icb = 2 * ip, 2 * ip + 1
                    ava = avpool.tile([C, IC], F32, tag="av_a")
                    avb = avpool.tile([C, IC], F32, tag="av_b")
                    for js in range(NJS):
                        if h == 0 and ip == 0 and js < NJS - VLEAD:
                            v_proj_one(js + VLEAD)
                        if h == 0 and ip in (1, 2) and js % 8 == 0:
                            qk_proj_one(
                                1,
                                4 * (ip - 1) + js // 8,
                                nc.vector.tensor_copy,
                                nc.vector.tensor_copy,
                            )
                        sim = simpool.tile([128, 1024], F32, tag="sim")
                        for t, ic in enumerate((ica, icb)):
                            nc.tensor.matmul(
                                sim[:, IC * t : IC * (t + 1)],
                                lhsT=kt[:, JS * js : JS * (js + 1)],
                                rhs=qt[:, IC * ic : IC * (ic + 1)],
                                start=True,
                                stop=True,
                            )
                        pt = pts.tile([128, 1024], BF16, tag="pt")
                        nc.scalar.activation(pt[:], sim[:], EXP)
                        for t, av in enumerate((ava, avb)):
                            nc.tensor.matmul(
                                av[:],
                                lhsT=vts[js][:, 33 * h : 33 * h + 128],
                                rhs=pt[:, IC * t : IC * (t + 1)],
                                start=(js == 0),
                                stop=(js == NJS - 1),
                            )
                    # ---- per-(head, i-chunk) epilogue
                    for ic, av in ((ica, ava), (icb, avb)):
                        sl = slice(hoff + IC * ic, hoff + IC * (ic + 1))
                        nc.vector.tensor_copy(avu[:, sl], av[0 : DH + 1, :])
                        nc.vector.reciprocal_approx_accurate(
                            rec[0:1, sl], avu[0:1, sl], scratch=rscr[0:1, :]
                        )
                        # broadcast 1/denom across partitions via a DRAM
                        # round-trip (DRAM APs allow 0-stride partition dims;
                        # keeps the PE out of the epilogue entirely)
                        slc = slice(IC * ic, IC * (ic + 1))
                        nc.sync.dma_start(out=recd[0:1, sl], in_=rec[0:1, sl])
                        dsl = recd[0:1, sl]
                        nc.sync.dma_start(
                            out=bc[:, slc],
                            in_=bass.AP(
                                tensor=dsl.tensor,
                                offset=dsl.offset,
                                ap=[[0, DH + 1]] + list(dsl.ap[1:]),
                            ),
                        )
                        nc.vector.tensor_mul(
                            avn[0 : DH + 1, sl], avu[:, sl], bc[:, slc]
                        )
                        if h == 1:
                            po = simpool.tile([128, 1024], F32, tag="sim")
                            nc.tensor.matmul(
                                po[:, 0:IC],
                                lhsT=woT_s[:, 0:128],
                                rhs=avn[:, IC * ic : IC * (ic + 1)],
                                start=True,
                                stop=False,
                            )
                            nc.tensor.matmul(
                                po[:, 0:IC],
                                lhsT=woT_s[:, 128:256],
                                rhs=avn[:, N + IC * ic : N + IC * (ic + 1)],
                                start=False,
                                stop=True,
                            )
                            # bias is folded into the projection (avn row 33
                            # is all-ones, woT row 33 of block 0 is bo)
                            nc.vector.tensor_copy(
                                outs[:, IC * ic : IC * (ic + 1)], po[:, 0:IC]
                            )
                            nc.sync.dma_start(
                                out=out_d[:, IC * ic : IC * (ic + 1)],
                                in_=outs[:, IC * ic : IC * (ic + 1)],
                            )
    nc.finalize()
    return nc


_nc_cache = None


def _get_nc():
    global _nc_cache
    if _nc_cache is None:
        _nc_cache = _build()
    return _nc_cache


def make_in_maps(x, wq, wk, wv, wo, bo):
    b = 4
    xt = np.asarray(x, np.float32).reshape(b, C, N)
    wq = np.asarray(wq, np.float32)
    wk = np.asarray(wk, np.float32)
    wv = np.asarray(wv, np.float32)
    wo = np.asarray(wo, np.float32)
    bo = np.asarray(bo, np.float32)
    scale = DH ** (-0.5)

    def bf(a):
        return np.ascontiguousarray(a.astype(ml_dtypes.bfloat16))

    in_maps = []
    for core in range(8):
        bi, hp = core // 2, core % 2
        wq2 = wq[64 * hp : 64 * hp + 64] * scale
        wk2 = wk[64 * hp : 64 * hp + 64]
        wv2 = wv[64 * hp : 64 * hp + 64]
        wqkT = np.zeros((C, 512), np.float32)
        wqkT[:, 0:32] = wq2.T[:, 0:32]  # Qh0
        wqkT[:, 128:160] = wq2.T[:, 32:64]  # Qh1
        wqkT[:, 256:288] = wk2.T[:, 0:32]  # Kh0
        wqkT[:, 384:416] = wk2.T[:, 32:64]  # Kh1
        wvT = np.zeros((C, VBLK), np.float32)  # cols 0,33 stay 0 (psum memset->1)
        wvT[:, 1:33] = wv2.T[:, 0:32]
        wvT[:, 34:66] = wv2.T[:, 32:64]
        woT = np.zeros((C, 256), np.float32)
        woT[1:33, 0:128] = wo[:, 64 * hp : 64 * hp + 32].T
        woT[1:33, 128:256] = wo[:, 64 * hp + 32 : 64 * hp + 64].T
        if hp == 0:
            woT[0, 0:128] = bo  # bias rides avn row 0 (= denom/denom = 1)
        in_maps.append(
            {
                "xt": bf(xt[bi]),
                "wqkT": bf(wqkT),
                "wvT": bf(wvT),
                "woT": bf(woT),
            }
        )
    return in_maps


def kernel(x, wq, wk, wv, wo, bo):
    global _last_results
    in_maps = make_in_maps(x, wq, wk, wv, wo, bo)
    nc = _get_nc()
    res = run_bass_kernel_spmd(nc, in_maps, core_ids=list(range(8)))
    _last_results = res
    outs = res.results
    out = np.zeros((4, C, N), np.float32)
    for bi in range(4):
        out[bi] = np.asarray(outs[2 * bi]["out"], np.float32) + np.asarray(
            outs[2 * bi + 1]["out"], np.float32
        )
    return out.reshape(4, C, 64, 64)


# revision 66
# speedup vs baseline: 1.1532x; 1.0366x over previous
"""Trainium2 8-core attention kernel for nn_Attention_14104672600564.

Problem: x[4,128,64,64] f32; wq/wk/wv/wo [128,128]; bo[128].
  per (b,h): sim = (wq x)^T (wk x) * d^-.5 ; attn = softmax(sim) ; out_h = attn @ (wv x)^T
  out = wo @ concat_h(out_h^T) + bo

Sharding: 16 independent (batch, head) attention problems -> 8 cores; each core
gets one batch and one head-pair. Each core computes its partial output
wo[:, headslice] @ heads_out [128, 4096]; the host unshards by summing the two
cores of each batch.

Perf design notes:
  - All matmuls bf16 (1 cyc/row; fp32 would be 4), accumulation fp32 in PSUM.
  - EVERY matmul is padded to untiled (128,128) PE mode (K and M padded to >64
    with zero rows/cols). Mixing PE tile modes forces a drain between
    matmuls: measured 630ns/matmul (isolated-cold) vs ~215ns pipelined.
  - softmax has no max-subtraction: |sim| < ~0.5 by construction.
  - exp runs on ACT at FD=1024 ([128,1024] PSUM->SBUF bf16), the intended
    bottleneck engine (~285us/core).
  - The AV matmul's stationary is the V^T block laced with ones columns, so
    output row 0 is the softmax denominator; reciprocal on DVE; broadcast of
    the reciprocal via a [128,128]-padded fp32 matmul whose stationary has a
    single row of ones.
"""

import sys

sys.path.insert(0, "/opt/trn_rl_repo")

import numpy as np
import ml_dtypes

import concourse.bass as bass
import concourse.bacc as bacc
import concourse.tile as tile
from concourse import mybir
import concourse.bass_utils as _bass_utils
from concourse.bass_utils import run_bass_kernel_spmd



BF16 = mybir.dt.bfloat16
F32 = mybir.dt.float32

HEADS = 4
DH = 32  # dim per head
C = 128  # channels
N = 4096  # tokens (64*64)
IC = 512  # i-chunk
NICH = N // IC  # 8
JS = 128  # j-strip
NJS = N // JS  # 32
VBLK = 2 * (DH + 1)  # 66: [1|Vh0|1|Vh1] per j-strip
VTW = VBLK * NJS + 33 + 128  # vt width incl. padding for the 128-wide lhsT AP

_last_results = None  # test harness pokes this for exec_time_ns / profile


def _build():
    nc = bacc.Bacc(None, target_bir_lowering=False)
    xt_d = nc.declare_dram_parameter("xt", [C, N], BF16, isOutput=False)
    wqkT_d = nc.declare_dram_parameter("wqkT", [C, 512], BF16, isOutput=False)
    wvT_d = nc.declare_dram_parameter("wvT", [C, VBLK], BF16, isOutput=False)
    woT_d = nc.declare_dram_parameter("woT", [C, 256], BF16, isOutput=False)
    out_d = nc.declare_dram_parameter("out", [C, N], F32, isOutput=True)
    recd = nc.dram_tensor("recd", [1, 2 * N], F32)  # reciprocal bounce for bcast

    EXP = mybir.ActivationFunctionType.Exp

    with tile.TileContext(nc) as tc:
        with (
            tc.tile_pool(name="singles", bufs=1) as singles,
            tc.tile_pool(name="pts", bufs=8) as pts,
            tc.tile_pool(name="simpool", bufs=3, space="PSUM") as simpool,
            tc.tile_pool(name="avpool", bufs=1, space="PSUM") as avpool,
        ):
            xt_s = singles.tile([C, N], BF16)
            wqkT_s = singles.tile([C, 512], BF16)
            wvT_s = singles.tile([C, VBLK], BF16)
            woT_s = singles.tile([C, 256], BF16)
            qt0 = singles.tile([C, N], BF16)  # head0 Q: rows 0-31, rest zero
            kt0 = singles.tile([C, N], BF16)  # head0 K
            qt1 = singles.tile([C, N], BF16)  # head1 Q
            kt1 = singles.tile([C, N], BF16)  # head1 K
            # one vt tile per j-strip: [1|Vh0|1|Vh1|zeros] padded to 161 cols
            # so both heads' 128-wide lhsT APs stay in untiled (128,128) mode,
            # and each AV matmul depends on exactly one V-projection.
            vts = [
                singles.tile([C, 161], BF16, tag=f"vt{j}", name=f"vt{j}")
                for j in range(NJS)
            ]
            avu = singles.tile([DH + 1, 2 * N], F32)
            avn = singles.tile([C, 2 * N], BF16)  # rows 33-127 zero
            rec = singles.tile([1, 2 * N], F32)  # 1/denom (full precision)
            rscr = singles.tile([1, IC], F32)  # reciprocal NR scratch
            bc = singles.tile([DH + 1, N], F32)  # broadcast recips (per-ic reuse)
            outs = singles.tile([C, N], F32)

            nc.sync.dma_start(out=wqkT_s[:], in_=wqkT_d[:])
            nc.sync.dma_start(out=wvT_s[:], in_=wvT_d[:])
            nc.sync.dma_start(out=woT_s[:], in_=woT_d[:])
            for j in range(NJS):
                nc.vector.memset(vts[j][:], 0.0)
            nc.vector.memset(avn[:], 0.0)
            for ic in range(NICH):
                nc.sync.dma_start(
                    out=xt_s[:, IC * ic : IC * (ic + 1)],
                    in_=xt_d[:, IC * ic : IC * (ic + 1)],
                )

            # ---- QK projection. wqkT is host-padded to [128, 512] with each
            # head-tensor's 32 columns at a 128-col stride and zeros elsewhere,
            # so every stationary slice is [W(32)|0(96)]: out rows 32-127 are
            # genuine zeros and the full [128,512] PSUM block lands in the qk
            # tile with no separate zero-fill. Evacuation copies run on ACT
            # (idle during the prologue; DVE was the prologue bottleneck).
            # Head0's copies go to ACT (fast prologue, exp stream starts right
            # after); head1's go to DVE and drain under head0's main loop.
            qts = [qt0, qt1]
            kts = [kt0, kt1]

            def qk_proj_one(h, ic, copy_q, copy_k):
                    ps = simpool.tile([128, 1024], F32, tag="sim")
                    for half in range(2):  # 0: Q, 1: K
                        c = 2 * half + h
                        nc.tensor.matmul(
                            ps[:, IC * half : IC * (half + 1)],
                            lhsT=wqkT_s[:, 128 * c : 128 * (c + 1)],
                            rhs=xt_s[:, IC * ic : IC * (ic + 1)],
                            start=True,
                            stop=True,
                        )
                    for half, eng, dst in ((0, copy_q, qts[h]), (1, copy_k, kts[h])):
                        eng(
                            dst[:, IC * ic : IC * (ic + 1)],
                            ps[:, IC * half : IC * (half + 1)],
                        )

            # head0 first (ACT copies: the exp stream starts right after);
            # head1's projection trickles into head0's second i-chunk pair.
            for ic in range(NICH):
                qk_proj_one(0, ic, nc.scalar.copy, nc.scalar.copy)

            # ---- V^T projection into the ones-laced layout.
            # wvT is host-padded to 66 cols with zeros at cols 0 and 33; the
            # projection writes [junk|Vh0|junk|Vh1] to PSUM, DVE memsets the
            # two junk columns to 1.0, then one contiguous copy (on ACT) lands
            # the whole block.
            def v_proj_one(jc):
                pv = simpool.tile([128, 1024], F32, tag="sim")
                nc.tensor.matmul(
                    pv[:, 0:VBLK],
                    lhsT=xt_s[:, JS * jc : JS * (jc + 1)],
                    rhs=wvT_s[:],
                    start=True,
                    stop=True,
                )
                nc.vector.memset(pv[:, 0:1], 1.0)
                nc.vector.memset(pv[:, 33:34], 1.0)
                nc.scalar.copy(vts[jc][:, 0:VBLK], pv[:, 0:VBLK])

            VLEAD = 6
            for jc in range(VLEAD):  # only the first strips gate the stream
                v_proj_one(jc)

            # ---- main attention loops (heads sequential).
            # Groups are (j-strip, i-chunk-pair): the two sim matmuls share one
            # stationary (the K strip) and the two AV matmuls share another
            # (the vt block), so each LDWEIGHTS serves two matmuls and
            # same-weight matmuls pipeline back-to-back on the PE.
            for h in range(2):
                qt, kt = qts[h], kts[h]
                hoff = N * h  # free offset into avu/avn/rec for this head
                for ip in range(NICH // 2):
                    ica, icb = 2 * ip, 2 * ip + 1
                    ava = avpool.tile([C, IC], F32, tag="av_a")
                    avb = avpool.tile([C, IC], F32, tag="av_b")
                    for js in range(NJS):
                        if h == 0 and ip == 0 and js < NJS - VLEAD:
                            v_proj_one(js + VLEAD)
                        if h == 0 and ip in (1, 2) and js % 8 == 0:
                            qk_proj_one(
                                1,
                                4 * (ip - 1) + js // 8,
                                nc.vector.tensor_copy,
                                nc.vector.tensor_copy,
                            )
                        sim = simpool.tile([128, 1024], F32, tag="sim")
                        for t, ic in enumerate((ica, icb)):
                            nc.tensor.matmul(
                                sim[:, IC * t : IC * (t + 1)],
                                lhsT=kt[:, JS * js : JS * (js + 1)],
                                rhs=qt[:, IC * ic : IC * (ic + 1)],
                                start=True,
                                stop=True,
                            )
                        pt = pts.tile([128, 1024], BF16, tag="pt")
                        nc.scalar.activation(pt[:], sim[:], EXP)
                        for t, av in enumerate((ava, avb)):
                            nc.tensor.matmul(
                                av[:],
                                lhsT=vts[js][:, 33 * h : 33 * h + 128],
                                rhs=pt[:, IC * t : IC * (t + 1)],
                                start=(js == 0),
                                stop=(js == NJS - 1),
                            )
                    # ---- per-(head, i-chunk) epilogue
                    for ic, av in ((ica, ava), (icb, avb)):
                        sl = slice(hoff + IC * ic, hoff + IC * (ic + 1))
                        nc.vector.tensor_copy(avu[:, sl], av[0 : DH + 1, :])
                        nc.vector.reciprocal_approx_accurate(
                            rec[0:1, sl], avu[0:1, sl], scratch=rscr[0:1, :]
                        )
                        # broadcast 1/denom across partitions via a DRAM
                        # round-trip (DRAM APs allow 0-stride partition dims;
                        # keeps the PE out of the epilogue entirely)
                        slc = slice(IC * ic, IC * (ic + 1))
                        nc.sync.dma_start(out=recd[0:1, sl], in_=rec[0:1, sl])
                        dsl = recd[0:1, sl]
                        nc.sync.dma_start(
                            out=bc[:, slc],
                            in_=bass.AP(
                                tensor=dsl.tensor,
                                offset=dsl.offset,
                                ap=[[0, DH + 1]] + list(dsl.ap[1:]),
                            ),
                        )
                        nc.vector.tensor_mul(
                            avn[0 : DH + 1, sl], avu[:, sl], bc[:, slc]
                        )
                        if h == 1:
                            # po borrows the evacuated AV slot (alternating
                            # tags): an AV-slot hold is absorbed by the 8-deep
                            # pt-buffer runway, a sim-slot hold stalls the exp
                            # stream directly.
                            po = avpool.tile(
                                [C, IC],
                                F32,
                                tag="av_a" if ic == ica else "av_b",
                                name=f"po{ic}",
                            )
                            nc.tensor.matmul(
                                po[:, 0:IC],
                                lhsT=woT_s[:, 0:128],
                                rhs=avn[:, IC * ic : IC * (ic + 1)],
                                start=True,
                                stop=False,
                            )
                            nc.tensor.matmul(
                                po[:, 0:IC],
                                lhsT=woT_s[:, 128:256],
                                rhs=avn[:, N + IC * ic : N + IC * (ic + 1)],
                                start=False,
                                stop=True,
                            )
                            # bias is folded into the projection (avn row 33
                            # is all-ones, woT row 33 of block 0 is bo)
                            nc.vector.tensor_copy(
                                outs[:, IC * ic : IC * (ic + 1)], po[:, 0:IC]
                            )
                            nc.sync.dma_start(
                                out=out_d[:, IC * ic : IC * (ic + 1)],
                                in_=outs[:, IC * ic : IC * (ic + 1)],
                            )
    nc.finalize()
    return nc


_nc_cache = None


def _get_nc():
    global _nc_cache
    if _nc_cache is None:
        _nc_cache = _build()
    return _nc_cache


def make_in_maps(x, wq, wk, wv, wo, bo):
    b = 4
    xt = np.asarray(x, np.float32).reshape(b, C, N)
    wq = np.asarray(wq, np.float32)
    wk = np.asarray(wk, np.float32)
    wv = np.asarray(wv, np.float32)
    wo = np.asarray(wo, np.float32)
    bo = np.asarray(bo, np.float32)
    scale = DH ** (-0.5)

    def bf(a):
        return np.ascontiguousarray(a.astype(ml_dtypes.bfloat16))

    in_maps = []
    for core in range(8):
        bi, hp = core // 2, core % 2
        wq2 = wq[64 * hp : 64 * hp + 64] * scale
        wk2 = wk[64 * hp : 64 * hp + 64]
        wv2 = wv[64 * hp : 64 * hp + 64]
        wqkT = np.zeros((C, 512), np.float32)
        wqkT[:, 0:32] = wq2.T[:, 0:32]  # Qh0
        wqkT[:, 128:160] = wq2.T[:, 32:64]  # Qh1
        wqkT[:, 256:288] = wk2.T[:, 0:32]  # Kh0
        wqkT[:, 384:416] = wk2.T[:, 32:64]  # Kh1
        wvT = np.zeros((C, VBLK), np.float32)  # cols 0,33 stay 0 (psum memset->1)
        wvT[:, 1:33] = wv2.T[:, 0:32]
        wvT[:, 34:66] = wv2.T[:, 32:64]
        woT = np.zeros((C, 256), np.float32)
        woT[1:33, 0:128] = wo[:, 64 * hp : 64 * hp + 32].T
        woT[1:33, 128:256] = wo[:, 64 * hp + 32 : 64 * hp + 64].T
        if hp == 0:
            woT[0, 0:128] = bo  # bias rides avn row 0 (= denom/denom = 1)
        in_maps.append(
            {
                "xt": bf(xt[bi]),
                "wqkT": bf(wqkT),
                "wvT": bf(wvT),
                "woT": bf(woT),
            }
        )
    return in_maps


def kernel(x, wq, wk, wv, wo, bo):
    global _last_results
    in_maps = make_in_maps(x, wq, wk, wv, wo, bo)
    nc = _get_nc()
    res = run_bass_kernel_spmd(nc, in_maps, core_ids=list(range(8)))
    _last_results = res
    outs = res.results
    out = np.zeros((4, C, N), np.float32)
    for bi in range(4):
        out[bi] = np.asarray(outs[2 * bi]["out"], np.float32) + np.asarray(
            outs[2 * bi + 1]["out"], np.float32
        )
    return out.reshape(4, C, 64, 64)
